# revision 2
# baseline (speedup 1.0000x reference)
"""Trainium2 Bass kernel for ContinuousConv1DSim (gnn_message_passing).

Reformulation (validated vs reference in fp32 numpy, rel err ~4e-5):
  G = F * npm (per-l mask), H = G * t
  MM1  (PE): psw[c2, l] = sum_j GH[j, c2] * Band[j, l]   -- causal 8-wide window
             sums over l, output TRANSPOSED (channels on partitions), with a
             second accumulating matmul adding the previous tile's halo rows.
  MM2a (PE): psp[l, 0:64]  = A_e   (window(G) @ W^T)
             psp[l, 64:128]= D_raw (window(H) @ W^T - window(G) @ bias)
  MM2b (PE): pssp[l, s*64+o] = u[s] * A_e[l, o]          -- s-expansion on PE
  sim_m   = (A_m * t - D_m) with A_m/D_m = npm * psp     (ACT copy w/ scale)
  obuf_sim= pssp * udt + sim_m (broadcast over s)        -- one DVE STT
  real[l] = npm[l] * (t[l] * A_m[l-1] - D_m[l-1])        -- partition-shifted STT
Output rows per l: [real, sim + u_s * udt * A] for s=0..7, last row real[L-1].

Pure data parallel: batch 32 -> 8 cores x 4. All params replicated.
"""

import numpy as np

B, L, C, O, S = 32, 2048, 64, 64, 8
NCORES = 8
BPC = B // NCORES          # 4 batches per core
NT = L // 128              # 16 l-tiles per batch
ROWS = (L - 1) * (S + 1) + 1  # 18424
F32 = None  # set after mybir import


def _consts(W, bias, u):
    n = np.arange(128)
    bandc = ((n[:, None] >= n[None, :] - 7) & (n[:, None] <= n[None, :])).astype(np.float32)
    bandp = (n[:, None] >= n[None, :] + 121).astype(np.float32)
    prba = np.zeros((128, 128), np.float32)
    prba[0:64, 0:64] = W.T           # A_e from U
    prba[0:64, 64:128] = -bias       # -F_e into D_raw
    prba[64:128, 64:128] = W.T       # TA_e into D_raw
    prbb = np.zeros((128, 512), np.float32)
    for s in range(S):
        prbb[0:64, s * 64:(s + 1) * 64] = u[s] * W.T
    return bandc, bandp, prba, prbb


def _build_nc():
    import concourse.bass as bass
    import concourse.bacc as bacc
    import concourse.mybir as mybir
    import concourse.tile as tile

    f32 = mybir.dt.float32
    Copy = mybir.ActivationFunctionType.Copy
    mult = mybir.AluOpType.mult
    sub = mybir.AluOpType.subtract
    add = mybir.AluOpType.add

    nc = bacc.Bacc("TRN2", target_bir_lowering=False, debug=False,
                   num_devices=NCORES)

    FD = nc.dram_tensor("f", [BPC, L, C], f32, kind="ExternalInput").ap()
    TSD = nc.dram_tensor("ts", [BPC, L + 128], f32, kind="ExternalInput").ap()
    UDD = nc.dram_tensor("ud", [BPC, L], f32, kind="ExternalInput").ap()
    NPD = nc.dram_tensor("np", [BPC, L + 128], f32, kind="ExternalInput").ap()
    BCD = nc.dram_tensor("bandc", [128, 128], f32, kind="ExternalInput").ap()
    BPD = nc.dram_tensor("bandp", [128, 128], f32, kind="ExternalInput").ap()
    PAD = nc.dram_tensor("prba", [128, 128], f32, kind="ExternalInput").ap()
    PBD = nc.dram_tensor("prbb", [128, 512], f32, kind="ExternalInput").ap()
    OUTD = nc.dram_tensor("out", [BPC, ROWS, O], f32, kind="ExternalOutput").ap()

    with tile.TileContext(nc) as tc:
        with (
            tc.tile_pool(name="const", bufs=1) as cpool,
            tc.tile_pool(name="scal", bufs=2) as spool,
            tc.tile_pool(name="feat", bufs=3) as fpool,
            tc.tile_pool(name="gh", bufs=3) as ghpool,
            tc.tile_pool(name="sbw", bufs=3) as sbwpool,
            tc.tile_pool(name="pp", bufs=3) as pppool,
            tc.tile_pool(name="simm", bufs=3) as simpool,
            tc.tile_pool(name="ob", bufs=3) as obpool,
            tc.tile_pool(name="ro", bufs=3) as ropool,
            tc.tile_pool(name="psw", bufs=3, space=bass.MemorySpace.PSUM) as pwpool,
            tc.tile_pool(name="psp", bufs=2, space=bass.MemorySpace.PSUM) as papool,
            tc.tile_pool(name="pssp", bufs=2, space=bass.MemorySpace.PSUM) as pbpool,
        ):
            bandc_t = cpool.tile([128, 128], f32, tag="bandc")
            bandp_t = cpool.tile([128, 128], f32, tag="bandp")
            prba_t = cpool.tile([128, 128], f32, tag="prba")
            prbb_t = cpool.tile([128, 512], f32, tag="prbb")
            zrow = cpool.tile([1, 64], f32, tag="zrow")
            nc.sync.dma_start(bandc_t[:], BCD)
            nc.sync.dma_start(bandp_t[:], BPD)
            nc.sync.dma_start(prba_t[:], PAD)
            nc.sync.dma_start(prbb_t[:], PBD)
            nc.gpsimd.memset(zrow[:], 0.0)

            for b in range(BPC):
                tst = spool.tile([128, NT], f32, tag="tst")
                tsh = spool.tile([128, NT], f32, tag="tsh")
                udt = spool.tile([128, NT], f32, tag="udt")
                npt = spool.tile([128, NT], f32, tag="npt")
                nsh = spool.tile([128, NT], f32, tag="nsh")
                nc.sync.dma_start(tst[:], TSD[b, 0:L].rearrange("(n p) -> p n", p=128))
                nc.sync.dma_start(tsh[:], TSD[b, 1:L + 1].rearrange("(n p) -> p n", p=128))
                nc.sync.dma_start(udt[:], UDD[b].rearrange("(n p) -> p n", p=128))
                nc.sync.dma_start(npt[:], NPD[b, 0:L].rearrange("(n p) -> p n", p=128))
                nc.sync.dma_start(nsh[:], NPD[b, 1:L + 1].rearrange("(n p) -> p n", p=128))
                # real row for l=0 is identically zero
                nc.sync.dma_start(OUTD[b, 0:1, :], zrow[:])

                psw_next = None
                for n in range(NT):
                    ftile = fpool.tile([128, C], f32, tag="f")
                    nc.sync.dma_start(ftile[:], FD[b, n * 128:(n + 1) * 128, :])
                    gh = ghpool.tile([128, 128], f32, tag="gh")
                    nc.scalar.activation(gh[:, 0:64], ftile[:], Copy,
                                         scale=npt[:, n:n + 1])
                    nc.vector.tensor_scalar_mul(gh[:, 64:128], gh[:, 0:64],
                                                tst[:, n:n + 1])
                    # MM1: windowed sums, transposed output
                    if n == 0:
                        psw_cur = pwpool.tile([128, 128], f32, tag="psw")
                        nc.tensor.matmul(psw_cur[:], gh[:], bandc_t[:],
                                         start=True, stop=True)
                    else:
                        psw_cur = psw_next
                        nc.tensor.matmul(psw_cur[:], gh[:], bandc_t[:],
                                         start=False, stop=True)
                    if n < NT - 1:
                        psw_next = pwpool.tile([128, 128], f32, tag="psw")
                        nc.tensor.matmul(psw_next[:], gh[:], bandp_t[:],
                                         start=True, stop=False)
                    sbw = sbwpool.tile([128, 128], f32, tag="sbw")
                    nc.scalar.copy(sbw[:], psw_cur[:])
                    # MM2: project windowed features
                    psp = papool.tile([128, 128], f32, tag="psp")
                    nc.tensor.matmul(psp[:], sbw[:], prba_t[:], start=True, stop=True)
                    pssp = pbpool.tile([128, 512], f32, tag="pssp")
                    nc.tensor.matmul(pssp[:], sbw[:], prbb_t[:], start=True, stop=True)
                    pp = pppool.tile([128, 128], f32, tag="pp")
                    nc.scalar.activation(pp[:], psp[:], Copy, scale=npt[:, n:n + 1])
                    sim_m = simpool.tile([128, 64], f32, tag="simm")
                    nc.vector.scalar_tensor_tensor(
                        sim_m[:], pp[:, 0:64], tst[:, n:n + 1], pp[:, 64:128],
                        op0=mult, op1=sub)
                    obsim = obpool.tile([128, 512], f32, tag="ob")
                    nc.vector.scalar_tensor_tensor(
                        obsim[:].rearrange("p (s o) -> p s o", o=64),
                        pssp[:].rearrange("p (s o) -> p s o", o=64),
                        udt[:, n:n + 1],
                        sim_m[:].unsqueeze(1).broadcast_to([128, 8, 64]),
                        op0=mult, op1=add)
                    # real rows for l = l0+1 .. l0+128, lane p -> l0+p+1
                    rr = ropool.tile([128, 64], f32, tag="rr")
                    nc.vector.scalar_tensor_tensor(
                        rr[:], pp[:, 0:64], tsh[:, n:n + 1], pp[:, 64:128],
                        op0=mult, op1=sub)
                    rm = ropool.tile([128, 64], f32, tag="rm")
                    nc.vector.tensor_scalar_mul(rm[:], rr[:], nsh[:, n:n + 1])
                    # store
                    PR = 128 if n < NT - 1 else 127
                    real_dst = bass.AP(
                        OUTD.tensor, (b * ROWS + 9 * (n * 128 + 1)) * 64,
                        [[9 * 64, PR], [1, 64]])
                    nc.sync.dma_start(real_dst, rm[0:PR, :])
                    if n < NT - 1:
                        blk = OUTD[b, 9 * n * 128: 9 * (n + 1) * 128, :] \
                            .rearrange("(p s) o -> p s o", s=9)
                        nc.sync.dma_start(blk[:, 1:9, :],
                                          obsim[:].rearrange("p (s o) -> p s o", o=64))
                    else:
                        blk = OUTD[b, 9 * n * 128: 9 * n * 128 + 9 * 127, :] \
                            .rearrange("(p s) o -> p s o", s=9)
                        nc.sync.dma_start(
                            blk[:, 1:9, :],
                            obsim[0:127, :].rearrange("p (s o) -> p s o", o=64))
    nc.compile()
    return nc


_NC_CACHE = None


def make_in_maps(inputs):
    times = np.ascontiguousarray(inputs["times"], np.float32)
    feats = np.ascontiguousarray(inputs["features"], np.float32)
    npm = inputs["non_pad_mask"].astype(np.float32)
    u = np.asarray(inputs["uniform_sample"], np.float32)
    W = np.ascontiguousarray(inputs["W"], np.float32)
    bias = np.ascontiguousarray(inputs["bias_param"], np.float32)

    bandc, bandp, prba, prbb = _consts(W, bias, u)
    tnext = np.concatenate([times[:, 1:], np.zeros((B, 1), np.float32)], 1)
    npmn = np.concatenate([npm[:, 1:], np.zeros((B, 1), np.float32)], 1)
    udt = (tnext - times) * npm * npmn  # (B, L); l=L-1 col unused downstream

    pad = np.zeros((B, 128), np.float32)
    times_p = np.concatenate([times, pad], 1)
    npm_p = np.concatenate([npm, pad], 1)

    in_maps = []
    for c in range(NCORES):
        sl = slice(c * BPC, (c + 1) * BPC)
        in_maps.append({
            "f": np.ascontiguousarray(feats[sl]),
            "ts": np.ascontiguousarray(times_p[sl]),
            "ud": np.ascontiguousarray(udt[sl]),
            "np": np.ascontiguousarray(npm_p[sl]),
            "bandc": bandc, "bandp": bandp, "prba": prba, "prbb": prbb,
        })
    return in_maps


def kernel(**inputs):
    global _NC_CACHE
    from concourse.bass_utils import run_bass_kernel_spmd

    if _NC_CACHE is None:
        _NC_CACHE = _build_nc()
    nc = _NC_CACHE

    in_maps = make_in_maps(inputs)
    res = run_bass_kernel_spmd(nc, in_maps, core_ids=list(range(NCORES)))
    out = np.concatenate([r["out"] for r in res.results], 0)
    return out.astype(np.float32)



# revision 4
# speedup vs baseline: 1.1207x; 1.1207x over previous
"""Trainium2 Bass kernel for ContinuousConv1DSim (gnn_message_passing).

Math (see reference): per l, window j in [l-7, l]:
  A[l,c]  = sum_j G[j,c]            (G = F * npm_j)
  D*[l,c] = sum_j (t_j - c_n) G[j,c]
  psp  = [A|D*] @ prbAD   -> A_e = A@W.T, D_raw = D*@W.T - A@bias
  pssp = [A|D*] @ prbU    -> u_s * A_e
  sim_m[l] = npm_l * ((t_l - c_n) A_e - D_raw)
  sim[l,s] = pssp * udt_l + sim_m
  real[l+1] = nsh_l * (sim_m + udr_l * A_e_masked)
Output row layout: lane l holds [sim[l,0..7], real[l+1]] -> DRAM rows
9l+1..9l+9 contiguous; row 0 (real[0]=0) is never written (outputs are
zero-initialized by the runtime; a zrow DMA writes it defensively).

Precision: G/H~ in fp16 with per-tile time centering c_n (halo handled by a
7-row second stationary with the next tile's center), window sums fp32 in
PSUM, MM2 in fp32r (~14-bit), output bf16 (host converts to fp32).

Sharding: pure data parallel, batch 32 -> 8 cores x 4.
"""

import numpy as np

B, L, C, O, S = 32, 2048, 64, 64, 8
NCORES = 8
BPC = B // NCORES
NT = L // 128
ROWS = (L - 1) * (S + 1) + 1  # 18424


def _consts(W, bias, u):
    n = np.arange(128)
    bandc = ((n[:, None] >= n[None, :] - 7) & (n[:, None] <= n[None, :])).astype(np.float32)
    # halo: row j of tile n contributes to col l of tile n+1 iff j >= l+121
    bandp7 = (n[121:128, None] >= n[None, :] + 121).astype(np.float32)  # [7,128]
    prb = np.zeros((128, 640), np.float32)
    for s in range(S):
        prb[0:64, s * 64:(s + 1) * 64] = u[s] * W.T
    prb[0:64, 512:576] = W.T            # A_e
    prb[0:64, 576:640] = -bias          # D_raw = D* @ W.T - A @ bias
    prb[64:128, 576:640] = W.T
    return bandc.astype(np.float16), bandp7.astype(np.float16), prb


def _host_prep(times, feats, npm):
    """Per-batch gh stationaries (fp16) and scalar columns (fp32)."""
    # centers per tile (index 0..NT; c[NT] only used by unreferenced slots)
    cent = np.zeros((B, NT + 1), np.float32)
    for n in range(NT):
        cent[:, n] = times[:, n * 128 + 63]
    cent[:, NT] = cent[:, NT - 1]

    G = feats * npm[:, :, None]                       # (B, L, C)
    tnext = np.concatenate([times[:, 1:], np.zeros((B, 1), np.float32)], 1)
    npmn = np.concatenate([npm[:, 1:], np.zeros((B, 1), np.float32)], 1)
    udr = tnext - times
    udt = udr * npm * npmn

    # gt: [B, 128, NT*128] fp16 : per tile [G | H_a] (H_a = G*(t - c_n))
    gt = np.zeros((B, 128, NT * 128), np.float16)
    # gb: [B, 7, NT*128] fp16 : rows 121..127 of [G | H_b] (H_b uses c_{n+1})
    gb = np.zeros((B, 7, NT * 128), np.float16)
    # sc: [B, 128, 5*NT] fp32 : [tstc | udr | udt | npt | nsh] per tile col
    sc = np.zeros((B, 128, 5 * NT), np.float32)
    for n in range(NT):
        sl = slice(n * 128, (n + 1) * 128)
        Gn = G[:, sl, :]                              # (B,128,C)
        tn = times[:, sl]                             # (B,128)
        ta = tn - cent[:, n][:, None]
        tb = tn - cent[:, n + 1][:, None]
        gt[:, :, n * 128:n * 128 + 64] = Gn.astype(np.float16)
        gt[:, :, n * 128 + 64:(n + 1) * 128] = (Gn * ta[:, :, None]).astype(np.float16)
        gb[:, :, n * 128:n * 128 + 64] = Gn[:, 121:128, :].astype(np.float16)
        gb[:, :, n * 128 + 64:(n + 1) * 128] = \
            (Gn[:, 121:128, :] * tb[:, 121:128, None]).astype(np.float16)
        sc[:, :, 0 * NT + n] = ta
        sc[:, :, 1 * NT + n] = udr[:, sl]
        sc[:, :, 2 * NT + n] = udt[:, sl]
        sc[:, :, 3 * NT + n] = npm[:, sl]
        sc[:, :, 4 * NT + n] = npmn[:, sl]
    return gt, gb, sc


def _build_nc():
    import concourse.bass as bass
    import concourse.bacc as bacc
    import concourse.mybir as mybir
    import concourse.tile as tile

    f32 = mybir.dt.float32
    f32r = mybir.dt.float32r
    bf16 = mybir.dt.bfloat16
    fp16 = mybir.dt.float16
    Copy = mybir.ActivationFunctionType.Copy
    mult = mybir.AluOpType.mult
    sub = mybir.AluOpType.subtract
    add = mybir.AluOpType.add

    nc = bacc.Bacc("TRN2", target_bir_lowering=False, debug=False,
                   num_devices=NCORES)

    GTD = nc.dram_tensor("gt", [BPC, 128, NT * 128], fp16, kind="ExternalInput").ap()
    GBD = nc.dram_tensor("gb", [BPC, 7, NT * 128], fp16, kind="ExternalInput").ap()
    SCD = nc.dram_tensor("sc", [BPC, 128, 5 * NT], f32, kind="ExternalInput").ap()
    BCD = nc.dram_tensor("bandc", [128, 128], fp16, kind="ExternalInput").ap()
    BPD = nc.dram_tensor("bandp7", [7, 128], fp16, kind="ExternalInput").ap()
    PRD = nc.dram_tensor("prb", [128, 640], f32r, kind="ExternalInput").ap()
    OUTD = nc.dram_tensor("out", [BPC, ROWS, O], bf16, kind="ExternalOutput").ap()

    with tile.TileContext(nc) as tc:
        with (
            tc.tile_pool(name="const", bufs=1) as cpool,
            tc.tile_pool(name="gt", bufs=2) as gtpool,
            tc.tile_pool(name="sbw", bufs=3) as sbwpool,
            tc.tile_pool(name="pp", bufs=3) as pppool,
            tc.tile_pool(name="simm", bufs=3) as simpool,
            tc.tile_pool(name="r0", bufs=3) as r0pool,
            tc.tile_pool(name="obp", bufs=2) as obppool,
            tc.tile_pool(name="ob", bufs=2) as obpool,
            tc.tile_pool(name="psw", bufs=3, space=bass.MemorySpace.PSUM) as pwpool,
            tc.tile_pool(name="psp", bufs=2, space=bass.MemorySpace.PSUM) as papool,
            tc.tile_pool(name="pssp", bufs=2, space=bass.MemorySpace.PSUM) as pbpool,
        ):
            bandc_t = cpool.tile([128, 128], fp16, tag="bandc")
            bandp_t = cpool.tile([7, 128], fp16, tag="bandp")
            prb_t = cpool.tile([128, 640], f32r, tag="prb")
            zrow = cpool.tile([1, 64], bf16, tag="zrow")
            nc.sync.dma_start(bandc_t[:], BCD)
            nc.sync.dma_start(bandp_t[:], BPD)
            nc.sync.dma_start(prb_t[:], PRD)
            nc.gpsimd.memset(zrow[:], 0.0)

            for b in range(BPC):
                gt = gtpool.tile([128, NT * 128], fp16, tag="gt")
                gb = gtpool.tile([7, NT * 128], fp16, tag="gb")
                sct = gtpool.tile([128, 5 * NT], f32, tag="sc")
                nc.sync.dma_start(gt[:], GTD[b])
                nc.sync.dma_start(gb[:], GBD[b])
                nc.sync.dma_start(sct[:], SCD[b])
                nc.sync.dma_start(OUTD[b, 0:1, :], zrow[:])

                def col(k, n):
                    return sct[:, k * NT + n:k * NT + n + 1]

                psw_next = None
                obuf = None
                for n in range(NT):
                    gh = gt[:, n * 128:(n + 1) * 128]
                    # MM1: window sums into psum (chained halo)
                    if n == 0:
                        psw_cur = pwpool.tile([128, 128], f32, tag="psw")
                        nc.tensor.matmul(psw_cur[:], gh, bandc_t[:],
                                         start=True, stop=True)
                    else:
                        psw_cur = psw_next
                        nc.tensor.matmul(psw_cur[:], gh, bandc_t[:],
                                         start=False, stop=True)
                    if n < NT - 1:
                        ghb = gb[:, n * 128:(n + 1) * 128]
                        psw_next = pwpool.tile([128, 128], f32, tag="psw")
                        nc.tensor.matmul(psw_next[:], ghb, bandp_t[:],
                                         start=True, stop=False)
                    # psum -> sbuf (fp32r) for MM2 stationary
                    sbw = sbwpool.tile([128, 128], f32r, tag="sbw")
                    if n % 2 == 0:
                        nc.vector.tensor_copy(sbw[:], psw_cur[:])
                    else:
                        nc.scalar.copy(sbw[:], psw_cur[:])
                    # MM2
                    psp = papool.tile([128, 128], f32, tag="psp")
                    nc.tensor.matmul(psp[:], sbw[:], prb_t[:, 512:640],
                                     start=True, stop=True)
                    pssp = pbpool.tile([128, 512], f32, tag="pssp")
                    nc.tensor.matmul(pssp[:], sbw[:], prb_t[:, 0:512],
                                     start=True, stop=True)
                    # masked projections
                    pp = pppool.tile([128, 128], f32, tag="pp")
                    nc.scalar.activation(pp[:], psp[:], Copy, scale=col(3, n))
                    # sim_m, real
                    simx = simpool.tile([128, 64], f32, tag="simx")
                    nc.gpsimd.tensor_scalar_mul(simx[:], pp[:, 0:64], col(0, n))
                    sim_m = simpool.tile([128, 64], bf16, tag="simm")
                    nc.gpsimd.tensor_tensor(sim_m[:], simx[:], pp[:, 64:128],
                                            op=sub)
                    r0 = r0pool.tile([128, 64], f32, tag="r0")
                    nc.vector.scalar_tensor_tensor(
                        r0[:], pp[:, 0:64], col(1, n), sim_m[:],
                        op0=mult, op1=add)
                    k = n % 4
                    if k == 0:
                        obuf = obpool.tile([128, 4 * 576], bf16, tag="ob")
                    nc.gpsimd.tensor_scalar_mul(
                        obuf[:, k * 576 + 512:(k + 1) * 576], r0[:], col(4, n))
                    # sim rows
                    simb = sim_m[:].unsqueeze(1).broadcast_to([128, 8, 64])
                    dst = obuf[:, k * 576:k * 576 + 512] \
                        .rearrange("p (s o) -> p s o", o=64)
                    if n % 2 == 0:
                        nc.vector.scalar_tensor_tensor(
                            dst, pssp[:].rearrange("p (s o) -> p s o", o=64),
                            col(2, n), simb, op0=mult, op1=add)
                    else:
                        obp = obppool.tile([128, 512], bf16, tag="obp")
                        nc.scalar.activation(obp[:], pssp[:], Copy,
                                             scale=col(2, n))
                        nc.vector.tensor_tensor(
                            dst, obp[:].rearrange("p (s o) -> p s o", o=64),
                            simb, op=add)
                    # quad DMA out
                    if k == 3:
                        q = n // 4
                        if n < NT - 1:
                            dram = bass.AP(
                                OUTD.tensor,
                                (b * ROWS + 9 * 512 * q + 1) * 64,
                                [[576, 128], [9 * 128 * 64, 4], [1, 576]])
                            src = obuf[:].rearrange("p (s o) -> p s o", o=576)
                            if q % 2 == 0:
                                nc.sync.dma_start(dram, src)
                            else:
                                nc.scalar.dma_start(dram, src)
                        else:
                            # last quad: lane 127 of tile 15 is out of range
                            dram_a = bass.AP(
                                OUTD.tensor,
                                (b * ROWS + 9 * 512 * q + 1) * 64,
                                [[576, 128], [9 * 128 * 64, 3], [1, 576]])
                            src_a = obuf[:, 0:3 * 576] \
                                .rearrange("p (s o) -> p s o", o=576)
                            nc.sync.dma_start(dram_a, src_a)
                            dram_b = bass.AP(
                                OUTD.tensor,
                                (b * ROWS + 9 * (512 * q + 384) + 1) * 64,
                                [[576, 127], [1, 576]])
                            nc.scalar.dma_start(dram_b, obuf[0:127, 3 * 576:4 * 576])
    nc.compile()
    return nc


_NC_CACHE = None


def make_in_maps(inputs):
    times = np.ascontiguousarray(inputs["times"], np.float32)
    feats = np.ascontiguousarray(inputs["features"], np.float32)
    npm = inputs["non_pad_mask"].astype(np.float32)
    u = np.asarray(inputs["uniform_sample"], np.float32)
    W = np.ascontiguousarray(inputs["W"], np.float32)
    bias = np.ascontiguousarray(inputs["bias_param"], np.float32)

    bandc, bandp7, prb = _consts(W, bias, u)
    gt, gb, sc = _host_prep(times, feats, npm)

    in_maps = []
    for c in range(NCORES):
        sl = slice(c * BPC, (c + 1) * BPC)
        in_maps.append({
            "gt": np.ascontiguousarray(gt[sl]),
            "gb": np.ascontiguousarray(gb[sl]),
            "sc": np.ascontiguousarray(sc[sl]),
            "bandc": bandc, "bandp7": bandp7, "prb": prb,
        })
    return in_maps


def kernel(**inputs):
    global _NC_CACHE
    from concourse.bass_utils import run_bass_kernel_spmd

    if _NC_CACHE is None:
        _NC_CACHE = _build_nc()
    nc = _NC_CACHE

    in_maps = make_in_maps(inputs)
    res = run_bass_kernel_spmd(nc, in_maps, core_ids=list(range(NCORES)))
    out = np.concatenate([np.asarray(r["out"]) for r in res.results], 0)
    return out.astype(np.float32)


# revision 5
# speedup vs baseline: 1.8213x; 1.6252x over previous
"""Trainium2 Bass kernel for ContinuousConv1DSim (gnn_message_passing).

Math (see reference): per l, window j in [l-7, l]:
  A[l,c]  = sum_j G[j,c]            (G = F * npm_j)
  D*[l,c] = sum_j (t_j - c_n) G[j,c]
  psp  = [A|D*] @ prbAD   -> A_e = A@W.T, D_raw = D*@W.T - A@bias
  pssp = [A|D*] @ prbU    -> u_s * A_e
  sim_m[l] = npm_l * ((t_l - c_n) A_e - D_raw)
  sim[l,s] = pssp * udt_l + sim_m
  real[l+1] = nsh_l * (sim_m + udr_l * A_e_masked)
Output row layout: lane l holds [sim[l,0..7], real[l+1]] -> DRAM rows
9l+1..9l+9 contiguous; row 0 (real[0]=0) is never written (outputs are
zero-initialized by the runtime; a zrow DMA writes it defensively).

Precision: G/H~ in fp16 with per-tile time centering c_n (halo handled by a
7-row second stationary with the next tile's center), window sums fp32 in
PSUM, MM2 in fp32r (~14-bit), output bf16 (host converts to fp32).

Sharding: pure data parallel, batch 32 -> 8 cores x 4.
"""

import numpy as np

B, L, C, O, S = 32, 2048, 64, 64, 8
NCORES = 8
BPC = B // NCORES
NT = L // 128
ROWS = (L - 1) * (S + 1) + 1  # 18424


def _consts(W, bias, u):
    n = np.arange(128)
    bandc = ((n[:, None] >= n[None, :] - 7) & (n[:, None] <= n[None, :])).astype(np.float32)
    # halo: row j of tile n contributes to col l of tile n+1 iff j >= l+121
    bandp7 = (n[121:128, None] >= n[None, :] + 121).astype(np.float32)  # [7,128]
    prb = np.zeros((128, 640), np.float32)
    for s in range(S):
        prb[0:64, s * 64:(s + 1) * 64] = u[s] * W.T
    prb[0:64, 512:576] = W.T            # A_e
    prb[0:64, 576:640] = -bias          # D_raw = D* @ W.T - A @ bias
    prb[64:128, 576:640] = W.T
    return bandc.astype(np.float16), bandp7.astype(np.float16), prb


def _host_prep(times, feats, npm):
    """Per-batch gh stationaries (fp16) and scalar columns (fp32)."""
    # centers per tile (index 0..NT; c[NT] only used by unreferenced slots)
    cent = np.zeros((B, NT + 1), np.float32)
    for n in range(NT):
        cent[:, n] = times[:, n * 128 + 63]
    cent[:, NT] = cent[:, NT - 1]

    G = feats * npm[:, :, None]                       # (B, L, C)
    tnext = np.concatenate([times[:, 1:], np.zeros((B, 1), np.float32)], 1)
    npmn = np.concatenate([npm[:, 1:], np.zeros((B, 1), np.float32)], 1)
    udr = tnext - times
    udt = udr * npm * npmn

    # gt: [B, 128, NT*128] fp16 : per tile [G | H_a] (H_a = G*(t - c_n))
    gt = np.zeros((B, 128, NT * 128), np.float16)
    # gb: [B, 7, NT*128] fp16 : rows 121..127 of [G | H_b] (H_b uses c_{n+1})
    gb = np.zeros((B, 7, NT * 128), np.float16)
    # sc: [B, 128, 5*NT] fp32 : [tstc | udr | udt | npt | nsh] per tile col
    sc = np.zeros((B, 128, 5 * NT), np.float32)
    for n in range(NT):
        sl = slice(n * 128, (n + 1) * 128)
        Gn = G[:, sl, :]                              # (B,128,C)
        tn = times[:, sl]                             # (B,128)
        ta = tn - cent[:, n][:, None]
        tb = tn - cent[:, n + 1][:, None]
        gt[:, :, n * 128:n * 128 + 64] = Gn.astype(np.float16)
        gt[:, :, n * 128 + 64:(n + 1) * 128] = (Gn * ta[:, :, None]).astype(np.float16)
        gb[:, :, n * 128:n * 128 + 64] = Gn[:, 121:128, :].astype(np.float16)
        gb[:, :, n * 128 + 64:(n + 1) * 128] = \
            (Gn[:, 121:128, :] * tb[:, 121:128, None]).astype(np.float16)
        sc[:, :, 0 * NT + n] = ta
        sc[:, :, 1 * NT + n] = udr[:, sl]
        sc[:, :, 2 * NT + n] = udt[:, sl]
        sc[:, :, 3 * NT + n] = npm[:, sl]
        sc[:, :, 4 * NT + n] = npmn[:, sl]
    return gt, gb, sc


def _build_nc():
    import concourse.bass as bass
    import concourse.bacc as bacc
    import concourse.mybir as mybir
    import concourse.tile as tile

    f32 = mybir.dt.float32
    f32r = mybir.dt.float32r
    bf16 = mybir.dt.bfloat16
    fp16 = mybir.dt.float16
    Copy = mybir.ActivationFunctionType.Copy
    mult = mybir.AluOpType.mult
    sub = mybir.AluOpType.subtract
    add = mybir.AluOpType.add

    nc = bacc.Bacc("TRN2", target_bir_lowering=False, debug=False,
                   num_devices=NCORES)

    GTD = nc.dram_tensor("gt", [BPC, 128, NT * 128], fp16, kind="ExternalInput").ap()
    GBD = nc.dram_tensor("gb", [BPC, 7, NT * 128], fp16, kind="ExternalInput").ap()
    SCD = nc.dram_tensor("sc", [BPC, 128, 5 * NT], f32, kind="ExternalInput").ap()
    BCD = nc.dram_tensor("bandc", [128, 128], fp16, kind="ExternalInput").ap()
    BPD = nc.dram_tensor("bandp7", [7, 128], fp16, kind="ExternalInput").ap()
    PRD = nc.dram_tensor("prb", [128, 640], f32r, kind="ExternalInput").ap()
    OUTD = nc.dram_tensor("out", [BPC, ROWS, O], bf16, kind="ExternalOutput").ap()

    with tile.TileContext(nc) as tc:
        with (
            tc.tile_pool(name="const", bufs=1) as cpool,
            tc.tile_pool(name="gt", bufs=2) as gtpool,
            tc.tile_pool(name="sbw", bufs=3) as sbwpool,
            tc.tile_pool(name="pp", bufs=3) as pppool,
            tc.tile_pool(name="simm", bufs=3) as simpool,
            tc.tile_pool(name="r0", bufs=3) as r0pool,
            tc.tile_pool(name="obp", bufs=2) as obppool,
            tc.tile_pool(name="ob", bufs=2) as obpool,
            tc.tile_pool(name="psw", bufs=3, space=bass.MemorySpace.PSUM) as pwpool,
            tc.tile_pool(name="psp", bufs=2, space=bass.MemorySpace.PSUM) as papool,
            tc.tile_pool(name="pssp", bufs=2, space=bass.MemorySpace.PSUM) as pbpool,
        ):
            bandc_t = cpool.tile([128, 128], fp16, tag="bandc")
            bandp_t = cpool.tile([7, 128], fp16, tag="bandp")
            prb_t = cpool.tile([128, 640], f32r, tag="prb")
            zrow = cpool.tile([1, 64], bf16, tag="zrow")
            nc.sync.dma_start(bandc_t[:], BCD)
            nc.sync.dma_start(bandp_t[:], BPD)
            nc.sync.dma_start(prb_t[:], PRD)
            nc.gpsimd.memset(zrow[:], 0.0)

            for b in range(BPC):
                gt = gtpool.tile([128, NT * 128], fp16, tag="gt")
                gb = gtpool.tile([7, NT * 128], fp16, tag="gb")
                sct = gtpool.tile([128, 5 * NT], f32, tag="sc")
                nc.sync.dma_start(gt[:], GTD[b])
                nc.sync.dma_start(gb[:], GBD[b])
                nc.sync.dma_start(sct[:], SCD[b])
                nc.sync.dma_start(OUTD[b, 0:1, :], zrow[:])

                def col(k, n):
                    return sct[:, k * NT + n:k * NT + n + 1]

                psw_next = None
                obuf = None
                for n in range(NT):
                    gh = gt[:, n * 128:(n + 1) * 128]
                    # MM1: window sums into psum (chained halo)
                    if n == 0:
                        psw_cur = pwpool.tile([128, 128], f32, tag="psw")
                        nc.tensor.matmul(psw_cur[:], gh, bandc_t[:],
                                         start=True, stop=True)
                    else:
                        psw_cur = psw_next
                        nc.tensor.matmul(psw_cur[:], gh, bandc_t[:],
                                         start=False, stop=True)
                    if n < NT - 1:
                        ghb = gb[:, n * 128:(n + 1) * 128]
                        psw_next = pwpool.tile([128, 128], f32, tag="psw")
                        nc.tensor.matmul(psw_next[:], ghb, bandp_t[:],
                                         start=True, stop=False)
                    # psum -> sbuf (fp32r) for MM2 stationary
                    sbw = sbwpool.tile([128, 128], f32r, tag="sbw")
                    if n % 2 == 0:
                        nc.vector.tensor_copy(sbw[:], psw_cur[:])
                    else:
                        nc.scalar.copy(sbw[:], psw_cur[:])
                    # MM2
                    psp = papool.tile([128, 128], f32, tag="psp")
                    nc.tensor.matmul(psp[:], sbw[:], prb_t[:, 512:640],
                                     start=True, stop=True)
                    pssp = pbpool.tile([128, 512], f32, tag="pssp")
                    nc.tensor.matmul(pssp[:], sbw[:], prb_t[:, 0:512],
                                     start=True, stop=True)
                    # masked projections
                    pp = pppool.tile([128, 128], f32, tag="pp")
                    nc.scalar.activation(pp[:], psp[:], Copy, scale=col(3, n))
                    # sim_m, real
                    simx = simpool.tile([128, 64], f32, tag="simx")
                    nc.vector.tensor_scalar_mul(simx[:], pp[:, 0:64], col(0, n))
                    sim_m = simpool.tile([128, 64], bf16, tag="simm")
                    nc.gpsimd.tensor_tensor(sim_m[:], simx[:], pp[:, 64:128],
                                            op=sub)
                    r0 = r0pool.tile([128, 64], f32, tag="r0")
                    nc.vector.scalar_tensor_tensor(
                        r0[:], pp[:, 0:64], col(1, n), sim_m[:],
                        op0=mult, op1=add)
                    k = n % 4
                    if k == 0:
                        obuf = obpool.tile([128, 4 * 576], bf16, tag="ob")
                    nc.vector.tensor_scalar_mul(
                        obuf[:, k * 576 + 512:(k + 1) * 576], r0[:], col(4, n))
                    # sim rows
                    simb = sim_m[:].unsqueeze(1).broadcast_to([128, 8, 64])
                    dst = obuf[:, k * 576:k * 576 + 512] \
                        .rearrange("p (s o) -> p s o", o=64)
                    if n % 2 == 0:
                        nc.vector.scalar_tensor_tensor(
                            dst, pssp[:].rearrange("p (s o) -> p s o", o=64),
                            col(2, n), simb, op0=mult, op1=add)
                    else:
                        obp = obppool.tile([128, 512], bf16, tag="obp")
                        nc.scalar.activation(obp[:], pssp[:], Copy,
                                             scale=col(2, n))
                        nc.vector.tensor_tensor(
                            dst, obp[:].rearrange("p (s o) -> p s o", o=64),
                            simb, op=add)
                    # quad DMA out
                    if k == 3:
                        q = n // 4
                        if n < NT - 1:
                            dram = bass.AP(
                                OUTD.tensor,
                                (b * ROWS + 9 * 512 * q + 1) * 64,
                                [[576, 128], [9 * 128 * 64, 4], [1, 576]])
                            src = obuf[:].rearrange("p (s o) -> p s o", o=576)
                            if q % 2 == 0:
                                nc.sync.dma_start(dram, src)
                            else:
                                nc.scalar.dma_start(dram, src)
                        else:
                            # last quad: lane 127 of tile 15 is out of range
                            dram_a = bass.AP(
                                OUTD.tensor,
                                (b * ROWS + 9 * 512 * q + 1) * 64,
                                [[576, 128], [9 * 128 * 64, 3], [1, 576]])
                            src_a = obuf[:, 0:3 * 576] \
                                .rearrange("p (s o) -> p s o", o=576)
                            nc.sync.dma_start(dram_a, src_a)
                            dram_b = bass.AP(
                                OUTD.tensor,
                                (b * ROWS + 9 * (512 * q + 384) + 1) * 64,
                                [[576, 127], [1, 576]])
                            nc.scalar.dma_start(dram_b, obuf[0:127, 3 * 576:4 * 576])
    nc.compile()
    return nc


_NC_CACHE = None


def make_in_maps(inputs):
    times = np.ascontiguousarray(inputs["times"], np.float32)
    feats = np.ascontiguousarray(inputs["features"], np.float32)
    npm = inputs["non_pad_mask"].astype(np.float32)
    u = np.asarray(inputs["uniform_sample"], np.float32)
    W = np.ascontiguousarray(inputs["W"], np.float32)
    bias = np.ascontiguousarray(inputs["bias_param"], np.float32)

    bandc, bandp7, prb = _consts(W, bias, u)
    gt, gb, sc = _host_prep(times, feats, npm)

    in_maps = []
    for c in range(NCORES):
        sl = slice(c * BPC, (c + 1) * BPC)
        in_maps.append({
            "gt": np.ascontiguousarray(gt[sl]),
            "gb": np.ascontiguousarray(gb[sl]),
            "sc": np.ascontiguousarray(sc[sl]),
            "bandc": bandc, "bandp7": bandp7, "prb": prb,
        })
    return in_maps


def kernel(**inputs):
    global _NC_CACHE
    from concourse.bass_utils import run_bass_kernel_spmd

    if _NC_CACHE is None:
        _NC_CACHE = _build_nc()
    nc = _NC_CACHE

    in_maps = make_in_maps(inputs)
    res = run_bass_kernel_spmd(nc, in_maps, core_ids=list(range(NCORES)))
    out = np.concatenate([np.asarray(r["out"]) for r in res.results], 0)
    return out.astype(np.float32)
